# revision 3
# baseline (speedup 1.0000x reference)
"""Multi-head attention + output Linear on 8 Trainium2 NeuronCores — v2.

Problem: bs=2, seq=2048, embed=1024, heads=16, head_dim=64.
  out = Linear(softmax(mask(Q K^T / 8)) V)        (eval-mode dropout)

Sharding: core c handles batch b = c//4 and query block qb = c%4 (512 query
rows), computing its exact [512, 1024] output slice — heads stay together per
core so the output Linear needs no cross-core reduction.

Per-core kernel (Tile framework), software-pipelined by one head (head h's
QK/exp/mask issue ahead of head h-1's PV/normalize/transpose):
  scoresT[k, q] = K_h Q_h^T            (f32r matmul, contraction d=64)
  probsT = exp(scoresT / 8) [fp16]     (ACT, PSUM->SBUF, [128,1024] tiles)
  probsT *= maskT                      (DVE fp16 2x mode, [128,2048] tiles)
  acc[q, 65] = sum_c probsT_c^T [V_c|1]  (PV with probs slices as weights:
                                          65-row streams instead of 512-row;
                                          ones column gives the denominator)
  attn[q, d] = acc[:, 0:64] * recip(acc[:, 64])  (denominator lives on the
                 q partition axis: one broadcast-AP DVE multiply per head)
  attnT[d, q] = PE-transpose(attn)     (identity-matmul transpose, fp16)
  y = attnT^T W^T + bias               (fp16 weights, fp32 PSUM accum)
Input DMAs are split/ordered so QK starts ~1.5us in; W/bias land last.
"""

import contextlib
import sys
import numpy as np

sys.path.insert(0, "/opt/trn_rl_repo")

import concourse.bass as bass
import concourse.tile as tile
from concourse import bacc, mybir
from concourse.bass_utils import run_bass_kernel_spmd

BS, SEQ, EMBED, HEADS = 2, 2048, 1024, 16
D = EMBED // HEADS            # 64
QB = SEQ // 4                 # 512 query rows per core
NC_COUNT = 8
KC = SEQ // 128               # 16 k chunks
F32 = mybir.dt.float32
F32R = mybir.dt.float32r
F16 = mybir.dt.float16

_CACHE = {}


def _build_nc(exp_group=2, scps_bufs=2, probs_bufs=10, kpool_bufs=3,
              sch_mod=0, pmask_stride=0, acc_bufs=2, tp_bufs=2,
              early_lin=0):
    nc = bacc.Bacc("TRN2", target_bir_lowering=False, debug=False)

    qT = nc.dram_tensor("qT", [8, 128, QB], F32R, kind="ExternalInput")
    kT = nc.dram_tensor("kT", [8, 128, SEQ], F32R, kind="ExternalInput")
    va = nc.dram_tensor("va", [HEADS, 128, KC, D + 1], F16, kind="ExternalInput")
    mT = nc.dram_tensor("mT", [128, KC, QB], F16, kind="ExternalInput")
    WT = nc.dram_tensor("WT", [128, 8, EMBED], F16, kind="ExternalInput")
    bias = nc.dram_tensor("bias", [EMBED], F32, kind="ExternalInput")
    ident = nc.dram_tensor("ident", [128, 128], F16, kind="ExternalInput")
    y = nc.dram_tensor("y", [QB, EMBED], F32, kind="ExternalOutput")

    G = exp_group
    ngrp = KC // G

    with tile.TileContext(nc) as tc, \
         nc.allow_low_precision(reason="fp16 probs/V/W with fp32 PSUM accumulate"):
        with tc.tile_pool(name="const", bufs=1) as const, \
             tc.tile_pool(name="kpool", bufs=kpool_bufs) as kpool, \
             tc.tile_pool(name="probs", bufs=probs_bufs) as probs, \
             tc.tile_pool(name="small", bufs=4) as small, \
             tc.tile_pool(name="ypool", bufs=4) as ypool, \
             tc.tile_pool(name="scps", bufs=scps_bufs, space="PSUM") as scps, \
             tc.tile_pool(name="accps", bufs=acc_bufs, space="PSUM") as accps, \
             tc.tile_pool(name="tpps", bufs=tp_bufs, space="PSUM") as tpps, \
             contextlib.ExitStack() as _es:
            ylin_pool = (_es.enter_context(
                tc.tile_pool(name="ylin", bufs=1, space="PSUM"))
                if early_lin else None)

            # ---- constants (ordered so the critical path starts ASAP:
            # kT[0]/qT[0]/first mask chunks/first heads' V go first; WT and
            # bias are only needed by the output linear at the very end) ----
            qT_sb = const.tile([128, 8, QB], F32R)
            mT_sb = const.tile([128, KC, QB], F16)
            va_sb = const.tile([128, HEADS, KC, D + 1], F16)
            WT_sb = const.tile([128, 8, EMBED], F16)
            ident_sb = const.tile([128, 128], F16)
            bias_bc = const.tile([128, EMBED], F32)
            attnT = const.tile([128, 8, QB], F16)

            kTiles = [kpool.tile([128, SEQ], F32R, tag="k", name=f"kT{hp}")
                      for hp in range(kpool_bufs)]
            nc.sync.dma_start(out=qT_sb[:, 0, :], in_=qT[0])
            nc.sync.dma_start(out=kTiles[0][:, 0:256], in_=kT[0][:, 0:256])
            nc.sync.dma_start(out=mT_sb[:, 0:2, :], in_=mT[:, 0:2, :])
            nc.sync.dma_start(out=kTiles[0][:, 256:768], in_=kT[0][:, 256:768])
            nc.sync.dma_start(out=mT_sb[:, 2:6, :], in_=mT[:, 2:6, :])
            nc.sync.dma_start(out=kTiles[0][:, 768:SEQ], in_=kT[0][:, 768:SEQ])
            for h in range(2):
                nc.sync.dma_start(out=va_sb[:, h], in_=va[h])
            nc.sync.dma_start(out=ident_sb, in_=ident[:, :])
            for i in range(1, kpool_bufs):
                nc.sync.dma_start(out=kTiles[i], in_=kT[i])
            nc.sync.dma_start(out=mT_sb[:, 6:KC, :], in_=mT[:, 6:KC, :])
            for hp in range(1, 8):
                nc.sync.dma_start(out=qT_sb[:, hp, :], in_=qT[hp])
            for h in range(2, HEADS):
                nc.sync.dma_start(out=va_sb[:, h], in_=va[h])
            nc.sync.dma_start(out=WT_sb, in_=WT[:, :, :])
            bias_ap = bias[:]
            nc.sync.dma_start(
                out=bias_bc,
                in_=bass.AP(tensor=bias_ap.tensor, offset=bias_ap.offset,
                            ap=[[0, 128]] + list(bias_ap.ap)),
            )

            def back_half(h, pes):
                """PV + normalize + transpose + evict for head h."""
                hp_, hh_ = h // 2, h % 2
                acc = accps.tile([128, 4, D + 1], F32, tag="acc", name=f"acc{h}")
                # PV: one accumulation chain per q-slice, run sequentially
                # (interleaved chains in one PSUM bank clobber each other at
                # start=True)
                for qs in range(4):
                    for c in range(KC):
                        nc.tensor.matmul(
                            acc[:, qs, :],
                            pes[c // (2 * G)][:, c % (2 * G),
                                              qs * 128:(qs + 1) * 128],
                            va_sb[:, h, c, :],
                            start=(c == 0), stop=(c == KC - 1))
                # normalize: per-partition (q) scalar multiply; the recip is
                # broadcast along d via a stride-0 AP so one op covers all 4
                # q-slices
                rc = small.tile([128, 4], F32, tag="rc", name=f"rc{h}")
                nc.vector.reciprocal(rc, acc[:, :, D])
                aq = small.tile([128, 4, D], F16, tag="aq", name=f"aq{h}")
                rc_ap = rc[:, :]
                rc_bc = bass.AP(tensor=rc_ap.tensor, offset=rc_ap.offset,
                                ap=list(rc_ap.ap) + [[0, D]])
                nc.vector.tensor_mul(aq, acc[:, :, 0:D], rc_bc)
                tp = tpps.tile([64, 4, 128], F16, tag="tp", name=f"tp{h}")
                for qs in range(4):
                    nc.tensor.transpose(tp[:, qs, :], aq[:, qs, :], ident_sb)
                nc.vector.tensor_copy(
                    attnT[hh_ * D:hh_ * D + D, hp_, :],
                    tp.rearrange("p a b -> p (a b)"))

            # ---- per-group engine roles ----
            # exp: ACT, or a Schraudolph bit-trick exp (int16(s*a+b) bitcast
            # as fp16 ~= exp(s/8)) on GPSIMD to offload the saturated ACT.
            # mask multiply: DVE, with some groups on GPSIMD.
            NG = 128
            sch_count, pm_count = sch_mod, pmask_stride
            cand = list(range(8, NG))  # keep head 0 pipeline warmup clean
            sch_set = set(cand[int(i * len(cand) / max(sch_count, 1))]
                          for i in range(sch_count)) if sch_count else set()
            # pool-mask works on probs PAIRS (4 chunks); pair index = gi // 2
            pcand = list(range(4, NG // 2))
            pmask_set = set(pcand[int(i * len(pcand) / max(pm_count, 1))]
                            for i in range(pm_count)) if pm_count else set()
            SCH_A = float(1024.0 * 1.4426950408889634 / np.sqrt(D))
            SCH_B = float(15360 - 60)

            gi = 0  # global group counter
            prev = None  # (head, probs tiles) software pipeline by one head
            snap13 = None  # scheduler priority snapshot after head 13 issued
            for hp in range(8):            # head pairs
                if hp < kpool_bufs:
                    kTp = kTiles[hp]
                else:
                    kTp = kpool.tile([128, SEQ], F32R, tag="k", name=f"kTp{hp}")
                    nc.sync.dma_start(out=kTp, in_=kT[hp])

                for hh in range(2):
                    h = 2 * hp + hh
                    if h == 15:
                        snap13 = tc.tile_snap_priority()
                    pes = []
                    pe_pair = None
                    masks = []
                    for g in range(ngrp):
                        sc = scps.tile([128, G, QB], F32, tag="sc")
                        for j in range(G):
                            c = g * G + j
                            nc.tensor.matmul(
                                sc[:, j, :],
                                kTp[hh * D:hh * D + D, c * 128:(c + 1) * 128],
                                qT_sb[hh * D:hh * D + D, hp, :],
                                start=True, stop=True)
                        # probs tiles span two groups (4 chunks) so one DVE
                        # mask multiply covers both
                        if g % 2 == 0:
                            pe_pair = probs.tile([128, 2 * G, QB], F16, tag="pe")
                            pes.append(pe_pair)
                        pe_t = pe_pair[:, (g % 2) * G:(g % 2) * G + G, :]
                        if gi in sch_set:
                            nc.vector.tensor_scalar(
                                pe_t.bitcast(mybir.dt.int16), sc,
                                SCH_A, SCH_B,
                                mybir.AluOpType.mult, mybir.AluOpType.add)
                        else:
                            nc.scalar.activation(
                                out=pe_t, in_=sc,
                                func=mybir.ActivationFunctionType.Exp,
                                scale=float(1.0 / np.sqrt(D)))
                        gi += 1
                        if g % 2 == 1:
                            # deferred: masks go behind the (slot-releasing)
                            # exp ops in the in-order DVE queue
                            masks.append((pe_pair, (gi - 1) // 2, g))
                    if prev is not None:
                        back_half(*prev)
                    for pe_pair, pi, g in masks:
                        eng = nc.gpsimd if pi in pmask_set else nc.vector
                        eng.tensor_mul(pe_pair, pe_pair,
                                       mT_sb[:, (g - 1) * G:(g + 1) * G, :])
                    prev = (h, pes)
            back_half(*prev)

            # ---- output linear ----
            # Early pass: kc 0..6 (available after head 13) issued with a
            # scheduler-priority shift into the head-14/15 region so the
            # chains fill PE slack under the ACT-bound attention tail; only
            # the kc=7 matmuls + adds remain exposed after the last head.
            y_sbs = []
            if ylin_pool is not None:
                off = tc.cur_priority - snap13
                with tc.high_priority(offset=off):
                    for qs in range(4):
                        y_sb = ypool.tile([128, EMBED], F32, tag="y",
                                          name=f"ysb{qs}")
                        y_sbs.append(y_sb)
                        for n in range(2):
                            yl = ylin_pool.tile([128, QB], F32, tag="yl",
                                                name=f"yl{qs}_{n}")
                            for kc in range(7):
                                nc.tensor.matmul(
                                    yl,
                                    attnT[:, kc, qs * 128:(qs + 1) * 128],
                                    WT_sb[:, kc, n * 512:(n + 1) * 512],
                                    start=(kc == 0), stop=(kc == 6))
                            nc.vector.tensor_add(
                                y_sb[:, n * 512:(n + 1) * 512], yl,
                                bias_bc[:, n * 512:(n + 1) * 512])
                for qs in range(4):
                    y_sb = y_sbs[qs]
                    for n in range(2):
                        yl = ylin_pool.tile([128, QB], F32, tag="yl",
                                            name=f"ylf{qs}_{n}")
                        nc.tensor.matmul(
                            yl, attnT[:, 7, qs * 128:(qs + 1) * 128],
                            WT_sb[:, 7, n * 512:(n + 1) * 512],
                            start=True, stop=True)
                        nc.vector.tensor_add(y_sb[:, n * 512:(n + 1) * 512],
                                             y_sb[:, n * 512:(n + 1) * 512],
                                             yl)
                    nc.sync.dma_start(out=y[qs * 128:(qs + 1) * 128, :],
                                      in_=y_sb)
            else:
                for qs in range(4):
                    y_sb = ypool.tile([128, EMBED], F32, tag="y",
                                      name=f"ysb{qs}")
                    for n in range(2):
                        ps = scps.tile([128, G, QB], F32, tag="sc")
                        for kc in range(8):
                            nc.tensor.matmul(
                                ps[:, 0, :],
                                attnT[:, kc, qs * 128:(qs + 1) * 128],
                                WT_sb[:, kc, n * 512:(n + 1) * 512],
                                start=(kc == 0), stop=(kc == 7))
                        nc.vector.tensor_add(y_sb[:, n * 512:(n + 1) * 512],
                                             ps[:, 0, :],
                                             bias_bc[:, n * 512:(n + 1) * 512])
                    nc.sync.dma_start(out=y[qs * 128:(qs + 1) * 128, :],
                                      in_=y_sb)

    nc.compile()
    return nc


def _prep_in_maps(q, k, v, padding_mask, W, b):
    q = np.asarray(q, dtype=np.float32)
    k = np.asarray(k, dtype=np.float32)
    v = np.asarray(v, dtype=np.float32)
    m = np.asarray(padding_mask)
    W = np.asarray(W, dtype=np.float32)
    b = np.asarray(b, dtype=np.float32)

    # q/k: [bs, seq, embed] -> [bs, 128(two heads' d), hp, seq-ish] layouts
    # qT host layout: [bs, 128, 8, seq]; per core slice cols qb*QB:(qb+1)*QB
    qr = q.reshape(BS, SEQ, 8, 128).transpose(0, 2, 3, 1)   # [bs, 8, 128, seq]
    kr = k.reshape(BS, SEQ, 8, 128).transpose(0, 2, 3, 1)   # [bs, 8, 128, seq]
    qr = np.ascontiguousarray(qr)
    kr = np.ascontiguousarray(kr)

    # va: [bs, 128, KC, HEADS, D+1] fp16 with ones column
    va = np.empty((BS, SEQ, HEADS, D + 1), dtype=np.float16)
    va[:, :, :, 0:D] = v.reshape(BS, SEQ, HEADS, D)
    va[:, :, :, D] = 1.0
    # [bs, (c p), h, dd] -> [bs, h, p, c, dd]
    va = np.ascontiguousarray(
        va.reshape(BS, KC, 128, HEADS, D + 1).transpose(0, 3, 2, 1, 4))

    # mask [bs, 1, q, k] -> [bs, 128, KC, q] fp16 (k-major partitions)
    mT = m[:, 0].transpose(0, 2, 1).astype(np.float16)       # [bs, k, q]
    mT = np.ascontiguousarray(
        mT.reshape(BS, KC, 128, SEQ).transpose(0, 2, 1, 3))  # [bs,128,KC,seq(q)]

    # WT: [128, 8, E] fp16 with WT[p, kc, e] = W[e, kc*128+p]
    WTc = np.ascontiguousarray(
        W.T.reshape(8, 128, EMBED).transpose(1, 0, 2)).astype(np.float16)

    ident = np.eye(128, dtype=np.float16)

    in_maps = []
    for c in range(NC_COUNT):
        bi, qb = c // 4, c % 4
        in_maps.append({
            "qT": np.ascontiguousarray(qr[bi, :, :, qb * QB:(qb + 1) * QB]),  # [8,128,QB]
            "kT": kr[bi],
            "va": va[bi],
            "mT": np.ascontiguousarray(mT[bi, :, :, qb * QB:(qb + 1) * QB]),
            "WT": WTc,
            "bias": b,
            "ident": ident,
        })
    return in_maps


def _run(in_maps, **kw):
    if "nc" not in _CACHE:
        _CACHE["nc"] = _build_nc()
    return run_bass_kernel_spmd(_CACHE["nc"], in_maps, list(range(NC_COUNT)), **kw)


def kernel(q, k, v, padding_mask, W, b):
    in_maps = _prep_in_maps(q, k, v, padding_mask, W, b)
    res = _run(in_maps)
    out = np.empty((BS, SEQ, EMBED), dtype=np.float32)
    for c in range(NC_COUNT):
        bi, qb = c // 4, c % 4
        out[bi, qb * QB:(qb + 1) * QB] = res.results[c]["y"]
    return out


# revision 4
# speedup vs baseline: 1.0058x; 1.0058x over previous
"""Multi-head attention + output Linear on 8 Trainium2 NeuronCores — v2.

Problem: bs=2, seq=2048, embed=1024, heads=16, head_dim=64.
  out = Linear(softmax(mask(Q K^T / 8)) V)        (eval-mode dropout)

Sharding: core c handles batch b = c//4 and query block qb = c%4 (512 query
rows), computing its exact [512, 1024] output slice — heads stay together per
core so the output Linear needs no cross-core reduction.

Per-core kernel (Tile framework), software-pipelined by one head (head h's
QK/exp/mask issue ahead of head h-1's PV/normalize/transpose):
  scoresT[k, q] = K_h Q_h^T            (f32r matmul, contraction d=64)
  probsT = exp(scoresT / 8) [fp16]     (ACT, PSUM->SBUF, [128,1024] tiles)
  probsT *= maskT                      (DVE fp16 2x mode, [128,2048] tiles)
  acc[q, 65] = sum_c probsT_c^T [V_c|1]  (PV with probs slices as weights:
                                          65-row streams instead of 512-row;
                                          ones column gives the denominator)
  attn[q, d] = acc[:, 0:64] * recip(acc[:, 64])  (denominator lives on the
                 q partition axis: one broadcast-AP DVE multiply per head)
  attnT[d, q] = PE-transpose(attn)     (identity-matmul transpose, fp16)
  y = attnT^T W^T + bias               (fp16 weights, fp32 PSUM accum)
Input DMAs are split/ordered so QK starts ~1.5us in; W/bias land last.
"""

import contextlib
import sys
import numpy as np

sys.path.insert(0, "/opt/trn_rl_repo")

import concourse.bass as bass
import concourse.tile as tile
from concourse import bacc, mybir
from concourse.bass_utils import run_bass_kernel_spmd

BS, SEQ, EMBED, HEADS = 2, 2048, 1024, 16
D = EMBED // HEADS            # 64
QB = SEQ // 4                 # 512 query rows per core
NC_COUNT = 8
KC = SEQ // 128               # 16 k chunks
F32 = mybir.dt.float32
F32R = mybir.dt.float32r
F16 = mybir.dt.float16

_CACHE = {}


def _build_nc(exp_group=2, scps_bufs=2, probs_bufs=10, kpool_bufs=3,
              sch_mod=0, pmask_stride=0, acc_bufs=2, tp_bufs=2,
              early_lin=0):
    nc = bacc.Bacc("TRN2", target_bir_lowering=False, debug=False)

    qT = nc.dram_tensor("qT", [8, 128, QB], F32R, kind="ExternalInput")
    kT = nc.dram_tensor("kT", [8, 128, SEQ], F32R, kind="ExternalInput")
    va = nc.dram_tensor("va", [HEADS, 128, KC, D + 1], F16, kind="ExternalInput")
    mT = nc.dram_tensor("mT", [128, KC, QB], F16, kind="ExternalInput")
    WT = nc.dram_tensor("WT", [128, 8, EMBED], F16, kind="ExternalInput")
    bias = nc.dram_tensor("bias", [EMBED], F32, kind="ExternalInput")
    ident = nc.dram_tensor("ident", [128, 128], F16, kind="ExternalInput")
    y = nc.dram_tensor("y", [QB, EMBED], F32, kind="ExternalOutput")

    G = exp_group
    ngrp = KC // G

    with tile.TileContext(nc) as tc, \
         nc.allow_low_precision(reason="fp16 probs/V/W with fp32 PSUM accumulate"):
        with tc.tile_pool(name="const", bufs=1) as const, \
             tc.tile_pool(name="kpool", bufs=kpool_bufs) as kpool, \
             tc.tile_pool(name="probs", bufs=probs_bufs) as probs, \
             tc.tile_pool(name="small", bufs=4) as small, \
             tc.tile_pool(name="ypool", bufs=4) as ypool, \
             tc.tile_pool(name="scps", bufs=scps_bufs, space="PSUM") as scps, \
             tc.tile_pool(name="accps", bufs=acc_bufs, space="PSUM") as accps, \
             tc.tile_pool(name="tpps", bufs=tp_bufs, space="PSUM") as tpps, \
             contextlib.ExitStack() as _es:
            ylin_pool = (_es.enter_context(
                tc.tile_pool(name="ylin", bufs=1, space="PSUM"))
                if early_lin else None)

            # ---- constants (ordered so the critical path starts ASAP:
            # kT[0]/qT[0]/first mask chunks/first heads' V go first; WT and
            # bias are only needed by the output linear at the very end) ----
            qT_sb = const.tile([128, 8, QB], F32R)
            mT_sb = const.tile([128, KC, QB], F16)
            va_sb = const.tile([128, HEADS, KC, D + 1], F16)
            WT_sb = const.tile([128, 8, EMBED], F16)
            ident_sb = const.tile([128, 128], F16)
            bias_bc = const.tile([128, EMBED], F32)
            attnT = const.tile([128, 8, QB], F16)

            kTiles = [kpool.tile([128, SEQ], F32R, tag="k", name=f"kT{hp}")
                      for hp in range(kpool_bufs)]
            nc.sync.dma_start(out=qT_sb[:, 0, :], in_=qT[0])
            nc.sync.dma_start(out=kTiles[0][:, 0:256], in_=kT[0][:, 0:256])
            nc.sync.dma_start(out=mT_sb[:, 0:2, :], in_=mT[:, 0:2, :])
            nc.sync.dma_start(out=kTiles[0][:, 256:768], in_=kT[0][:, 256:768])
            nc.sync.dma_start(out=mT_sb[:, 2:6, :], in_=mT[:, 2:6, :])
            nc.sync.dma_start(out=kTiles[0][:, 768:SEQ], in_=kT[0][:, 768:SEQ])
            for h in range(2):
                nc.sync.dma_start(out=va_sb[:, h], in_=va[h])
            nc.sync.dma_start(out=ident_sb, in_=ident[:, :])
            for i in range(1, kpool_bufs):
                nc.sync.dma_start(out=kTiles[i], in_=kT[i])
            nc.sync.dma_start(out=mT_sb[:, 6:KC, :], in_=mT[:, 6:KC, :])
            for hp in range(1, 8):
                nc.sync.dma_start(out=qT_sb[:, hp, :], in_=qT[hp])
            for h in range(2, HEADS):
                nc.sync.dma_start(out=va_sb[:, h], in_=va[h])

            def back_half(h, pes):
                """PV + normalize + transpose + evict for head h."""
                hp_, hh_ = h // 2, h % 2
                acc = accps.tile([128, 4, D + 1], F32, tag="acc", name=f"acc{h}")
                # PV: one accumulation chain per q-slice, run sequentially
                # (interleaved chains in one PSUM bank clobber each other at
                # start=True)
                for qs in range(4):
                    for c in range(KC):
                        nc.tensor.matmul(
                            acc[:, qs, :],
                            pes[c // (2 * G)][:, c % (2 * G),
                                              qs * 128:(qs + 1) * 128],
                            va_sb[:, h, c, :],
                            start=(c == 0), stop=(c == KC - 1))
                # normalize: per-partition (q) scalar multiply; the recip is
                # broadcast along d via a stride-0 AP so one op covers all 4
                # q-slices
                rc = small.tile([128, 4], F32, tag="rc", name=f"rc{h}")
                nc.vector.reciprocal(rc, acc[:, :, D])
                aq = small.tile([128, 4, D], F16, tag="aq", name=f"aq{h}")
                rc_ap = rc[:, :]
                rc_bc = bass.AP(tensor=rc_ap.tensor, offset=rc_ap.offset,
                                ap=list(rc_ap.ap) + [[0, D]])
                nc.vector.tensor_mul(aq, acc[:, :, 0:D], rc_bc)
                tp = tpps.tile([64, 4, 128], F16, tag="tp", name=f"tp{h}")
                for qs in range(4):
                    nc.tensor.transpose(tp[:, qs, :], aq[:, qs, :], ident_sb)
                nc.vector.tensor_copy(
                    attnT[hh_ * D:hh_ * D + D, hp_, :],
                    tp.rearrange("p a b -> p (a b)"))

            # ---- per-group engine roles ----
            # exp: ACT, or a Schraudolph bit-trick exp (int16(s*a+b) bitcast
            # as fp16 ~= exp(s/8)) on GPSIMD to offload the saturated ACT.
            # mask multiply: DVE, with some groups on GPSIMD.
            NG = 128
            sch_count, pm_count = sch_mod, pmask_stride
            cand = list(range(8, NG))  # keep head 0 pipeline warmup clean
            sch_set = set(cand[int(i * len(cand) / max(sch_count, 1))]
                          for i in range(sch_count)) if sch_count else set()
            # pool-mask works on probs PAIRS (4 chunks); pair index = gi // 2
            pcand = list(range(4, NG // 2))
            pmask_set = set(pcand[int(i * len(pcand) / max(pm_count, 1))]
                            for i in range(pm_count)) if pm_count else set()
            SCH_A = float(1024.0 * 1.4426950408889634 / np.sqrt(D))
            SCH_B = float(15360 - 60)

            gi = 0  # global group counter
            prev = None  # (head, probs tiles) software pipeline by one head
            snap13 = None  # scheduler priority snapshot after head 13 issued
            for hp in range(8):            # head pairs
                if hp < kpool_bufs:
                    kTp = kTiles[hp]
                else:
                    kTp = kpool.tile([128, SEQ], F32R, tag="k", name=f"kTp{hp}")
                    nc.sync.dma_start(out=kTp, in_=kT[hp])
                if hp == 5:
                    nc.sync.dma_start(out=WT_sb, in_=WT[:, :, :])
                    bias_ap = bias[:]
                    nc.sync.dma_start(
                        out=bias_bc,
                        in_=bass.AP(tensor=bias_ap.tensor,
                                    offset=bias_ap.offset,
                                    ap=[[0, 128]] + list(bias_ap.ap)),
                    )

                for hh in range(2):
                    h = 2 * hp + hh
                    if h == 15:
                        snap13 = tc.tile_snap_priority()
                    pes = []
                    pe_pair = None
                    masks = []
                    for g in range(ngrp):
                        sc = scps.tile([128, G, QB], F32, tag="sc")
                        for j in range(G):
                            c = g * G + j
                            nc.tensor.matmul(
                                sc[:, j, :],
                                kTp[hh * D:hh * D + D, c * 128:(c + 1) * 128],
                                qT_sb[hh * D:hh * D + D, hp, :],
                                start=True, stop=True)
                        # probs tiles span two groups (4 chunks) so one DVE
                        # mask multiply covers both
                        if g % 2 == 0:
                            pe_pair = probs.tile([128, 2 * G, QB], F16, tag="pe")
                            pes.append(pe_pair)
                        pe_t = pe_pair[:, (g % 2) * G:(g % 2) * G + G, :]
                        if gi in sch_set:
                            nc.vector.tensor_scalar(
                                pe_t.bitcast(mybir.dt.int16), sc,
                                SCH_A, SCH_B,
                                mybir.AluOpType.mult, mybir.AluOpType.add)
                        else:
                            nc.scalar.activation(
                                out=pe_t, in_=sc,
                                func=mybir.ActivationFunctionType.Exp,
                                scale=float(1.0 / np.sqrt(D)))
                        gi += 1
                        if g % 2 == 1:
                            # deferred: masks go behind the (slot-releasing)
                            # exp ops in the in-order DVE queue
                            masks.append((pe_pair, (gi - 1) // 2, g))
                    if prev is not None:
                        back_half(*prev)
                    for pe_pair, pi, g in masks:
                        eng = nc.gpsimd if pi in pmask_set else nc.vector
                        eng.tensor_mul(pe_pair, pe_pair,
                                       mT_sb[:, (g - 1) * G:(g + 1) * G, :])
                    prev = (h, pes)
            back_half(*prev)

            # ---- output linear ----
            # Early pass: kc 0..6 (available after head 13) issued with a
            # scheduler-priority shift into the head-14/15 region so the
            # chains fill PE slack under the ACT-bound attention tail; only
            # the kc=7 matmuls + adds remain exposed after the last head.
            y_sbs = []
            if ylin_pool is not None:
                off = tc.cur_priority - snap13
                with tc.high_priority(offset=off):
                    for qs in range(4):
                        y_sb = ypool.tile([128, EMBED], F32, tag="y",
                                          name=f"ysb{qs}")
                        y_sbs.append(y_sb)
                        for n in range(2):
                            yl = ylin_pool.tile([128, QB], F32, tag="yl",
                                                name=f"yl{qs}_{n}")
                            for kc in range(7):
                                nc.tensor.matmul(
                                    yl,
                                    attnT[:, kc, qs * 128:(qs + 1) * 128],
                                    WT_sb[:, kc, n * 512:(n + 1) * 512],
                                    start=(kc == 0), stop=(kc == 6))
                            nc.vector.tensor_add(
                                y_sb[:, n * 512:(n + 1) * 512], yl,
                                bias_bc[:, n * 512:(n + 1) * 512])
                for qs in range(4):
                    y_sb = y_sbs[qs]
                    for n in range(2):
                        yl = ylin_pool.tile([128, QB], F32, tag="yl",
                                            name=f"ylf{qs}_{n}")
                        nc.tensor.matmul(
                            yl, attnT[:, 7, qs * 128:(qs + 1) * 128],
                            WT_sb[:, 7, n * 512:(n + 1) * 512],
                            start=True, stop=True)
                        nc.vector.tensor_add(y_sb[:, n * 512:(n + 1) * 512],
                                             y_sb[:, n * 512:(n + 1) * 512],
                                             yl)
                    nc.sync.dma_start(out=y[qs * 128:(qs + 1) * 128, :],
                                      in_=y_sb)
            else:
                for qs in range(4):
                    y_sb = ypool.tile([128, EMBED], F32, tag="y",
                                      name=f"ysb{qs}")
                    for n in range(2):
                        ps = scps.tile([128, G, QB], F32, tag="sc")
                        for kc in range(8):
                            nc.tensor.matmul(
                                ps[:, 0, :],
                                attnT[:, kc, qs * 128:(qs + 1) * 128],
                                WT_sb[:, kc, n * 512:(n + 1) * 512],
                                start=(kc == 0), stop=(kc == 7))
                        nc.vector.tensor_add(y_sb[:, n * 512:(n + 1) * 512],
                                             ps[:, 0, :],
                                             bias_bc[:, n * 512:(n + 1) * 512])
                        nc.sync.dma_start(
                            out=y[qs * 128:(qs + 1) * 128,
                                  n * 512:(n + 1) * 512],
                            in_=y_sb[:, n * 512:(n + 1) * 512])

    nc.compile()
    return nc


def _prep_in_maps(q, k, v, padding_mask, W, b):
    q = np.asarray(q, dtype=np.float32)
    k = np.asarray(k, dtype=np.float32)
    v = np.asarray(v, dtype=np.float32)
    m = np.asarray(padding_mask)
    W = np.asarray(W, dtype=np.float32)
    b = np.asarray(b, dtype=np.float32)

    # q/k: [bs, seq, embed] -> [bs, 128(two heads' d), hp, seq-ish] layouts
    # qT host layout: [bs, 128, 8, seq]; per core slice cols qb*QB:(qb+1)*QB
    qr = q.reshape(BS, SEQ, 8, 128).transpose(0, 2, 3, 1)   # [bs, 8, 128, seq]
    kr = k.reshape(BS, SEQ, 8, 128).transpose(0, 2, 3, 1)   # [bs, 8, 128, seq]
    qr = np.ascontiguousarray(qr)
    kr = np.ascontiguousarray(kr)

    # va: [bs, 128, KC, HEADS, D+1] fp16 with ones column
    va = np.empty((BS, SEQ, HEADS, D + 1), dtype=np.float16)
    va[:, :, :, 0:D] = v.reshape(BS, SEQ, HEADS, D)
    va[:, :, :, D] = 1.0
    # [bs, (c p), h, dd] -> [bs, h, p, c, dd]
    va = np.ascontiguousarray(
        va.reshape(BS, KC, 128, HEADS, D + 1).transpose(0, 3, 2, 1, 4))

    # mask [bs, 1, q, k] -> [bs, 128, KC, q] fp16 (k-major partitions)
    mT = m[:, 0].transpose(0, 2, 1).astype(np.float16)       # [bs, k, q]
    mT = np.ascontiguousarray(
        mT.reshape(BS, KC, 128, SEQ).transpose(0, 2, 1, 3))  # [bs,128,KC,seq(q)]

    # WT: [128, 8, E] fp16 with WT[p, kc, e] = W[e, kc*128+p]
    WTc = np.ascontiguousarray(
        W.T.reshape(8, 128, EMBED).transpose(1, 0, 2)).astype(np.float16)

    ident = np.eye(128, dtype=np.float16)

    in_maps = []
    for c in range(NC_COUNT):
        bi, qb = c // 4, c % 4
        in_maps.append({
            "qT": np.ascontiguousarray(qr[bi, :, :, qb * QB:(qb + 1) * QB]),  # [8,128,QB]
            "kT": kr[bi],
            "va": va[bi],
            "mT": np.ascontiguousarray(mT[bi, :, :, qb * QB:(qb + 1) * QB]),
            "WT": WTc,
            "bias": b,
            "ident": ident,
        })
    return in_maps


def _run(in_maps, **kw):
    if "nc" not in _CACHE:
        _CACHE["nc"] = _build_nc()
    return run_bass_kernel_spmd(_CACHE["nc"], in_maps, list(range(NC_COUNT)), **kw)


def kernel(q, k, v, padding_mask, W, b):
    in_maps = _prep_in_maps(q, k, v, padding_mask, W, b)
    res = _run(in_maps)
    out = np.empty((BS, SEQ, EMBED), dtype=np.float32)
    for c in range(NC_COUNT):
        bi, qb = c // 4, c % 4
        out[bi, qb * QB:(qb + 1) * QB] = res.results[c]["y"]
    return out
